# revision 68
# baseline (speedup 1.0000x reference)
"""Causal self-attention Trainium2 kernel (8 NeuronCores, tensor-parallel heads).

Problem: B=4, T=2048, C=1024, H=16, D=64 fp32.
  q,k,v = x@Wq, x@Wk, x@Wv  (biases are zeros by spec)
  per-head causal softmax(qk^T/8) @ v, then @ Wp (+bp on host) summed over cores.

Sharding: 2 heads per core (column-split Wq/Wk/Wv, row-split Wp). Each core
computes a partial output [B*T, C] in bf16; host sums the 8 partials in fp32
and adds bp.

Per-core dataflow (all matmul operands bf16, fp32 PSUM accumulation):
  xT [C, B*T] bf16 streamed from DRAM (host pre-transposes + casts x).
  Q^T/K^T [128, B*T] = w.T @ xT  (w slices [C,128] as stationary, FWL-eligible).
  V natural [tk,2,64] per 128-key tile via ONE [128,128] PE transpose of the
  V^T tile (both heads at once); ones column appended -> V_aug [tk, 65] so
  P@V_aug also yields softmax row sums.
  S^T [tk, N'] = K^T.T @ Q^T per (batch, tq-block, tk-tile); the two heads are
  K=64 matmuls on distinct PE row groups into one 2-bank PSUM tile. Diagonal
  tiles are trimmed to queries [off, 512) (off = key-offset within the block);
  the remaining 128-wide triangle is zeroed on DVE by multiplying P with a
  0/1 mask after exp. No max-subtraction pass (scores are O(1), exp in range).
  P^T = exp(S^T) via one double-wide ACT op per tile, PSUM -> SBUF bf16.
  Y_aug^T [65, N'] = V_aug.T @ P^T accumulated over tk tiles; row 64 is the
  softmax denominator. Normalize: reciprocal_approx_fast directly from PSUM ->
  gpsimd partition_broadcast -> DVE multiply -> Y^T [128, RT] per block.
  out_part [512, 1024] = Y^T.T @ Wp_slice per block, staged to SBUF bf16 and
  DMA'd out as computed.
"""

import numpy as np

import concourse.tile as tile
from concourse import bacc, mybir
from concourse.bass_utils import run_bass_kernel_spmd

F32 = mybir.dt.float32
BF16 = mybir.dt.bfloat16

B, T, C, H = 4, 2048, 1024, 16
D = C // H  # 64
N_CORES = 8
RT = 512  # row-tile (tq block) size
KT = C // 128  # 8 contraction tiles for projections
DEBUG_TAPS = False


def build_kernel(n_batches=B):
    nc = bacc.Bacc(None, target_bir_lowering=False, debug=False)
    rows = n_batches * T
    bt_rt = T // RT  # 4 tq blocks per batch

    xT_d = nc.dram_tensor("xT", [128, rows // RT, KT, RT], BF16, kind="ExternalInput")
    wq_d = nc.dram_tensor("wq", [128, KT, 128], BF16, kind="ExternalInput")
    wk_d = nc.dram_tensor("wk", [128, KT, 128], BF16, kind="ExternalInput")
    wv_d = nc.dram_tensor("wv", [128, KT, 128], BF16, kind="ExternalInput")
    wp_d = nc.dram_tensor("wp", [128, C], BF16, kind="ExternalInput")
    mk_d = nc.dram_tensor("mask2", [128, 2, 128], BF16, kind="ExternalInput")
    id_d = nc.dram_tensor("ident", [128, 128], BF16, kind="ExternalInput")
    on_d = nc.dram_tensor("onescol", [128, 2 * (T // 128)], BF16, kind="ExternalInput")
    out_d = nc.dram_tensor("out", [rows, C], BF16, kind="ExternalOutput")
    if DEBUG_TAPS:
        dbgq_d = nc.dram_tensor("dbgq", [128, RT], BF16, kind="ExternalOutput")
        dbgv_d = nc.dram_tensor("dbgv", [128, 2, T // 128, 65], BF16, kind="ExternalOutput")
        dbgpt_d = nc.dram_tensor("dbgpt", [128, 2, RT], BF16, kind="ExternalOutput")
        dbgyt_d = nc.dram_tensor("dbgyt", [128, RT], BF16, kind="ExternalOutput")
        dbgys_d = nc.dram_tensor("dbgys", [65, 2, RT], F32, kind="ExternalOutput")
        dbgbc_d = nc.dram_tensor("dbgbc", [64, 2, RT], F32, kind="ExternalOutput")

    with tile.TileContext(nc) as tc:
        with (
            nc.allow_low_precision(reason="bf16 intermediates are intentional"),
            tc.tile_pool(name="const", bufs=1) as const,
            tc.tile_pool(name="big", bufs=1) as big,
            tc.tile_pool(name="xs", bufs=3) as xs,
            tc.tile_pool(name="vt", bufs=2) as vtp,
            tc.tile_pool(name="pt", bufs=4) as ptp,
            tc.tile_pool(name="yt", bufs=4) as ytp,
            tc.tile_pool(name="nrm", bufs=3) as nrm,
            tc.tile_pool(name="ob", bufs=2) as ob,
            tc.tile_pool(name="psS", bufs=2, space="PSUM") as psS,
            tc.tile_pool(name="psY", bufs=2, space="PSUM") as psY,
            tc.tile_pool(name="psA", bufs=1, space="PSUM") as psA,
            tc.tile_pool(name="psO", bufs=1, space="PSUM") as psO,
        ):
            # ---- warmup: seed an SBUF tile via engine memset (no DMA
            # dependency) and run matmuls on it immediately so the PE HAM
            # un-throttles to 2.4GHz before the first real projection.
            wtile = const.tile([128, 256], BF16, name="wtile")
            nc.vector.memset(wtile[:], 1.0)
            warm = psA.tile([128, 512], F32, name="warm", tag="a")
            for _ in range(60):
                nc.tensor.matmul(
                    warm[:, 0:256],
                    wtile[:, 0:128],
                    wtile[:],
                    start=True,
                    stop=True,
                )

            mask2 = const.tile([128, 2, 128], BF16)
            nc.sync.dma_start(mask2[:], mk_d[:])
            ident = const.tile([128, 128], BF16)
            nc.gpsimd.dma_start(ident[:], id_d[:])

            wq = const.tile([128, KT, 128], BF16)
            wk = const.tile([128, KT, 128], BF16)
            wv = const.tile([128, KT, 128], BF16)
            wp = const.tile([128, C], BF16)

            # ---- whole-run big buffers ----
            n_rt_all = rows // RT
            qTs = [big.tile([128, RT], BF16, name=f"qT{i}") for i in range(n_rt_all)]
            kTs = [big.tile([128, RT], BF16, name=f"kT{i}") for i in range(n_rt_all)]
            n_vt = T // 128  # 16 v-tiles per batch
            v_augs = [
                big.tile([128, 2, n_vt, 65], BF16, name=f"vaug{i}") for i in range(2)
            ]
            for va in v_augs:
                nc.gpsimd.memset(va[:, :, :, 64:65], 1.0)

            xt0 = xs.tile([128, KT, RT], BF16, name="xt")
            for k in range(0, 2):
                nc.sync.dma_start(xt0[:, k, :], xT_d[:, 0, k, :])
            nc.sync.dma_start(wq[:, 0:2, :], wq_d[:, 0:2, :])
            for k in range(2, KT):
                nc.sync.dma_start(xt0[:, k, :], xT_d[:, 0, k, :])
            nc.sync.dma_start(wq[:, 2:8, :], wq_d[:, 2:8, :])
            nc.sync.dma_start(wk[:], wk_d[:])
            nc.sync.dma_start(wv[:], wv_d[:])
            nc.sync.dma_start(wp[:], wp_d[:])

            pending_oproj = []

            def emit_oproj_unit(yt, q0, rr, nn, final=False):
                if final and (rr * 2 + nn) % 2 == 1:
                    ops = psS.tile([128, 512], F32, name="opsf", tag="s")
                else:
                    ops = psO.tile([128, 512], F32, name="ops", tag="o")
                nc.tensor.matmul(
                    ops[:],
                    yt[:, rr * 128 : rr * 128 + 128],
                    wp[:, nn * 512 : nn * 512 + 512],
                    start=True,
                    stop=True,
                )
                osb = ob.tile([128, 512], BF16, name="osb")
                nc.vector.tensor_copy(osb[:], ops[:])
                nc.sync.dma_start(
                    out_d[
                        q0 + rr * 128 : q0 + rr * 128 + 128,
                        nn * 512 : nn * 512 + 512,
                    ],
                    osb[:],
                )

            def emit_oproj(yt, q0, final=False):
                for rr in range(RT // 128):
                    for nn in range(C // 512):
                        emit_oproj_unit(yt, q0, rr, nn, final=final)

            def make_proj_steps(b):
                """Projection work for batch b as a list of closures, each a
                self-contained chunk of PE/DVE work, so attention loops can
                interleave them into program order as PE filler."""
                r0 = b * T
                v_aug = v_augs[b % 2]
                holder = {}
                steps = []

                def load(rt):
                    if b == 0 and rt == 0:
                        holder[rt] = xt0
                        return
                    rtg = (r0 + rt * RT) // RT
                    xt = xs.tile([128, KT, RT], BF16, name="xt")
                    for kh in range(0, KT, 2):
                        nc.sync.dma_start(
                            xt[:, kh : kh + 2, :], xT_d[:, rtg, kh : kh + 2, :]
                        )
                    holder[rt] = xt

                def chain(rt, wi):
                    rtg = (r0 + rt * RT) // RT
                    xt = holder[rt]
                    w = (wq, wk, wv)[wi]
                    acc = psA.tile([128, RT], F32, name="proj", tag="a")
                    for k in range(KT):
                        nc.tensor.matmul(
                            acc[:],
                            w[:, k, :],
                            xt[:, k, :],
                            start=(k == 0),
                            stop=(k == KT - 1),
                        )
                    if wi == 0:
                        nc.vector.tensor_copy(qTs[rtg][:], acc[:])
                    elif wi == 1:
                        nc.vector.tensor_copy(kTs[rtg][:], acc[:])
                    else:
                        vt_sb = vtp.tile([128, RT], BF16, name="vt_sb")
                        nc.vector.tensor_copy(vt_sb[:], acc[:])
                        for c in range(RT // 128):
                            vtile = rt * (RT // 128) + c
                            vps = psO.tile([128, 2, 64], BF16, name="vps", tag="o")
                            nc.tensor.transpose(
                                vps[:], vt_sb[:, c * 128 : c * 128 + 128], ident[:]
                            )
                            nc.vector.tensor_copy(v_aug[:, :, vtile, 0:64], vps[:])

                for rt in range(bt_rt):
                    steps.append(lambda rt=rt: load(rt))
                    for wi in range(3):
                        steps.append(lambda rt=rt, wi=wi: chain(rt, wi))
                return steps

            with nc.named_scope("proj0"):
                for s in make_proj_steps(0):
                    s()

            for b in range(n_batches):
                r0 = b * T
                if DEBUG_TAPS and b == 0:
                    nc.sync.dma_start(dbgq_d[:], qTs[0][:])
                    nc.sync.dma_start(dbgv_d[:], v_augs[0][:])
                # Weave the NEXT batch's projection chains between this
                # batch's attention tiles: keeps the PE dense while the
                # ACT engine grinds the exp of each tile.
                steps = make_proj_steps(b + 1) if b + 1 < n_batches else []
                emitted = 0
                tiles_done = 0
                total_tiles = sum((t + 1) * (RT // 128) for t in range(bt_rt))
                oq = []  # outproj units woven one per attention tile
                for tqb in range(bt_rt):
                    with nc.named_scope(f"attn{b}_{tqb}"):
                        if pending_oproj:
                            yt_p, q0_p = pending_oproj.pop()
                            for rr in range(RT // 128):
                                for nn in range(C // 512):
                                    oq.append(
                                        lambda yt=yt_p, q0=q0_p, rr=rr, nn=nn:
                                        emit_oproj_unit(yt, q0, rr, nn)
                                    )
                        q0 = r0 + tqb * RT
                        n_tk = (tqb + 1) * (RT // 128)
                        yps = [
                            psY.tile([65, RT], F32, name=f"yacc{h}", tag="y")
                            for h in range(2)
                        ]
                        v_aug = v_augs[b % 2]

                        def emit_pv(pt, off, tk):
                            for h in range(2):
                                nc.tensor.matmul(
                                    yps[h][:, off:RT],
                                    v_aug[:, h, tk, :],
                                    pt[:, h, off:RT],
                                    start=(tk == 0),
                                    stop=(tk == n_tk - 1),
                                    skip_group_check=True,
                                )

                        pvq = []
                        for tk in range(n_tk):
                            k0 = r0 + tk * 128
                            diag = tk * 128 >= tqb * RT
                            off = (tk - tqb * (RT // 128)) * 128 if diag else 0
                            st = psS.tile([128, 2, RT], F32, name="st", tag="s")
                            kt_tile = kTs[k0 // RT]
                            kk = k0 % RT
                            qt_tile = qTs[q0 // RT]
                            for h in range(2):
                                hs = slice(64 * h, 64 * h + 64)
                                nc.tensor.matmul(
                                    st[:, h, off:RT],
                                    kt_tile[hs, kk : kk + 128],
                                    qt_tile[hs, off:RT],
                                    start=True,
                                    stop=True,
                                    skip_group_check=True,
                                )
                            pt = ptp.tile([128, 2, RT], BF16, name="pt")
                            nc.scalar.activation(
                                pt[:, :, off:RT],
                                st[:, :, off:RT],
                                mybir.ActivationFunctionType.Exp,
                            )
                            if diag:
                                nc.vector.tensor_mul(
                                    pt[:, :, off : off + 128],
                                    pt[:, :, off : off + 128],
                                    mask2[:],
                                )
                            if DEBUG_TAPS and b == 0 and tqb == 1 and tk == 0:
                                nc.sync.dma_start(dbgpt_d[:], pt[:])
                            # software pipeline: PV lags QK by 2 tiles so the
                            # in-order PE never head-blocks on exp + semaphore
                            # latency while later QKs are ready.
                            pvq.append((pt, off, tk))
                            if len(pvq) > 2:
                                emit_pv(*pvq.pop(0))
                            tiles_done += 1
                            if oq:
                                oq.pop(0)()
                            while emitted < len(steps) * tiles_done // total_tiles:
                                steps[emitted]()
                                emitted += 1
                        while pvq:
                            emit_pv(*pvq.pop(0))
                        # ---- normalize -> Y^T block [128, RT] ----
                        yt = ytp.tile([128, RT], BF16, name="yt")
                        for h in range(2):
                            ssum = nrm.tile([1, RT], F32, name="ssum")
                            nc.vector.tensor_copy(ssum[:], yps[h][64:65, :])
                            srow = nrm.tile([1, RT], F32, name="srow")
                            nc.vector.reciprocal_approx_fast(srow[:], ssum[:])
                            bc = nrm.tile([64, RT], F32, name="bc")
                            nc.gpsimd.partition_broadcast(bc[:], srow[:])
                            nc.vector.tensor_mul(
                                yt[64 * h : 64 * h + 64, :], yps[h][0:64, :], bc[:]
                            )
                            if DEBUG_TAPS and b == 0 and tqb == 1:
                                ysb = ob.tile([65, RT], F32, name="ysb")
                                nc.vector.tensor_copy(ysb[:], yps[h][:])
                                nc.sync.dma_start(dbgys_d[:, h, :], ysb[:])
                                bsb = ob.tile([64, RT], F32, name="bsb")
                                nc.vector.tensor_copy(bsb[:], bc[:])
                                nc.sync.dma_start(dbgbc_d[:, h, :], bsb[:])
                        if DEBUG_TAPS and b == 0 and tqb == 1:
                            nc.sync.dma_start(dbgyt_d[:], yt[:])
                    pending_oproj.append((yt, q0))
                while oq:
                    oq.pop(0)()
                while emitted < len(steps):
                    steps[emitted]()
                    emitted += 1
            while pending_oproj:
                emit_oproj(*pending_oproj.pop(), final=True)
    nc.compile()
    return nc


def make_mask():
    """mask2[p, h, jj] = 1.0 iff jj >= p (valid causal entry within the
    128-wide diagonal triangle), duplicated for both heads."""
    m = (np.arange(128)[None, :] >= np.arange(128)[:, None]).astype(np.float32)
    return np.ascontiguousarray(np.broadcast_to(m[:, None, :], (128, 2, 128)))


def _wlayout(W, cols, scale=1.0):
    """[C, 128] slice -> [128 p, KT k, 128 m] with element (p,k,m) = W[k*128+p, m]."""
    import ml_dtypes

    w = (np.asarray(W, np.float32) * scale)[:, cols]  # [1024, 128]
    return np.ascontiguousarray(
        w.reshape(KT, 128, 128).transpose(1, 0, 2)
    ).astype(ml_dtypes.bfloat16)


def make_inputs_for_core(c, shared, Wq, Wk, Wv, Wp):
    import ml_dtypes

    cols = slice(c * 128, (c + 1) * 128)
    bf = ml_dtypes.bfloat16
    d = dict(shared)
    d["wq"] = _wlayout(Wq, cols, 1.0 / 8.0)
    d["wk"] = _wlayout(Wk, cols)
    d["wv"] = _wlayout(Wv, cols)
    d["wp"] = np.ascontiguousarray(np.asarray(Wp, np.float32)[cols, :]).astype(bf)
    return d


def kernel(x, Wq, bq, Wk, bk, Wv, bv, Wp, bp, _nc_cache={}, **run_kwargs):
    import ml_dtypes

    n_batches = B
    if "nc" not in _nc_cache:
        _nc_cache["nc"] = build_kernel(n_batches)
    nc = _nc_cache["nc"]
    bf = ml_dtypes.bfloat16
    # [128 p, n_rt rt, KT k, RT r] with element (p,rt,k,r) = x[rt*RT+r, k*128+p]
    xT = np.ascontiguousarray(
        np.asarray(x, np.float32)
        .reshape(B * T // RT, RT, KT, 128)
        .transpose(3, 0, 2, 1)
    ).astype(bf)
    shared = {
        "xT": xT,
        "mask2": make_mask().astype(bf),
        "ident": np.eye(128, dtype=np.float32).astype(bf),
        "onescol": np.ones((128, 2 * (T // 128)), np.float32).astype(bf),
    }
    in_maps = [make_inputs_for_core(c, shared, Wq, Wk, Wv, Wp) for c in range(N_CORES)]
    res = run_bass_kernel_spmd(nc, in_maps, core_ids=list(range(N_CORES)), **run_kwargs)
    out = np.zeros((B * T, C), np.float32)
    for r in res.results:
        out += np.asarray(r["out"], np.float32)
    out += np.asarray(bp, np.float32)[None, :]
    if run_kwargs.get("trace"):
        kernel.last_result = res
    return out.reshape(B, T, C)
